# revision 65
# baseline (speedup 1.0000x reference)
"""Multi-head attention (B=4, L=2048, D=1024, H=16) on 8 Trainium2 NeuronCores.

Sharding: core c handles batch b=c//2 and head-half half=c%2 (8 heads = 512 of
the 1024 projection output dims).  Each core computes its heads' Q/K/V
projections, attention, and a full-L partial of the output projection
(contracting only its 512 head dims).  The host sums the two partials per batch
(the "all-reduce after fc" done at gather time).

Single fused pipeline: the 256-iteration attention stream (4 head-pairs x
4 lq-blocks x 16 lk-tiles) is emitted as one continuous instruction stream.
S^T matmuls run two iterations ahead of the AV/denominator matmuls so the
scalar engine's exp activations stream back-to-back; K/Q projections of later
pairs and output-projection groups are interleaved one matmul per iteration as
filler so the tensor engine never idles (holding its high p-state).

PSUM (8 banks): 2 x st[128,1024]f32 (4) + 2 x proj[128,512]f32 (2) +
av[128,512]f32 (1) + dn[128,512]f32 (1).
"""

import sys
from collections import deque

import numpy as np

if "/opt/trn_rl_repo" not in sys.path:
    sys.path.insert(0, "/opt/trn_rl_repo")

import concourse.bass as bass
import concourse.mybir as mybir
from concourse import bacc
import concourse.tile as tile
from concourse.bass import ts

F32 = mybir.dt.float32
F16 = mybir.dt.float16
I16 = mybir.dt.int16
EXP = mybir.ActivationFunctionType.Exp
# Schraudolph fp16 exp: int16(s * 1024/ln2 + (15360 - 44)) viewed as fp16
SCH_A = 1024.0 / float(np.log(2.0))
SCH_B = 15360.0 - 44.0
# Offload exp of iterations gi%SCH_MOD==2 to the vector engine (0 = off)
SCH_MOD = 0

L = 2048          # sequence length
D = 1024          # model dim
OC = 512          # output-projection dims owned by one core (8 heads x 64)
NPAIR = 4         # head pairs per core (pair = 128 projection dims)
NLQB = 4          # lq blocks of 512
LQB = 512
NLK = 16          # lk tiles of 128
ND = 8            # d-model tiles of 128
N_CORES = 8
NITER = NPAIR * NLQB * NLK  # 256


def build_program():
    nc = bacc.Bacc("TRN2", debug=False, enable_asserts=False,
                   target_bir_lowering=False)

    # Inputs arrive HOST-PRE-TILED so every DMA source is contiguous per
    # partition (4-8KB descriptor runs instead of 0.5-1KB): qT/kT as
    # [slice, p, n, 512], vT as [vblk, p, n, 256], weights as [p, n, cols].
    qT = nc.dram_tensor("qT", [4, 128, ND, 512], F16,
                        kind="ExternalInput").ap()
    kT = nc.dram_tensor("kT", [4, 128, ND, 512], F16,
                        kind="ExternalInput").ap()
    vT = nc.dram_tensor("vT", [8, 128, ND, 256], F16,
                        kind="ExternalInput").ap()
    wqT = nc.dram_tensor("wqT", [128, ND, OC], F16,
                         kind="ExternalInput").ap()
    wkT = nc.dram_tensor("wkT", [128, ND, OC], F16,
                         kind="ExternalInput").ap()
    wvT = nc.dram_tensor("wvT", [128, ND, OC], F16,
                         kind="ExternalInput").ap()
    woT = nc.dram_tensor("woT", [128, NPAIR, D], F16,
                         kind="ExternalInput").ap()
    out = nc.dram_tensor("out", [L, D], F32, kind="ExternalOutput").ap()

    with tile.TileContext(nc, pool_alloc_mode="queue") as tc:
        build_body(nc, tc, qT, kT, vT, wqT, wkT, wvT, woT, out)
    nc.compile()
    return nc


def build_body(nc, tc, qT, kT, vT, wqT, wkT, wvT, woT, out):
    # ---- pools ----------------------------------------------------------
    st_pool = tc.alloc_tile_pool(name="st", bufs=2, space="PSUM")   # 4 banks
    pj_pool = tc.alloc_tile_pool(name="pj", bufs=2, space="PSUM")   # 2 banks
    av_pool = tc.alloc_tile_pool(name="av", bufs=1, space="PSUM")   # 1 bank
    dn_pool = tc.alloc_tile_pool(name="dn", bufs=1, space="PSUM")   # 1 bank

    kh_pool = tc.alloc_tile_pool(name="kh", bufs=3)
    qh_pool = tc.alloc_tile_pool(name="qh", bufs=3)
    pt_pool = tc.alloc_tile_pool(name="pt", bufs=7)
    pa_pool = tc.alloc_tile_pool(name="pa", bufs=4)
    rc_pool = tc.alloc_tile_pool(name="rc", bufs=2)
    oc_pool = tc.alloc_tile_pool(name="ocp", bufs=2)

    ones_t, _free_ones = tc.tile([128, 64], F16, name="ones_t")
    nc.vector.memset(ones_t[:], 1.0)
    wrm_t, _free_wrm = tc.tile([128, 512], F16, name="wrm_t")
    nc.vector.memset(wrm_t[:], 0.0)

    # Warm the tensor engine while the first input DMAs are in flight: its
    # clock ramps only under sustained execution (2.4GHz after ~3us of
    # continuous busy), so ~6us of full-width throwaway matmuls ahead of the
    # V projection lets real work start at full speed.  N must be large --
    # narrow matmuls are decode-bound and barely accumulate busy time.
    for wu in range(18):
        psb = pj_pool.tile([128, 512], F32, tag="pj", name=f"wu{wu}")
        nc.tensor.matmul(psb[0:64, :], ones_t[:], wrm_t[:],
                         start=True, stop=True)

    vh_t, _free_vh = tc.tile([128, NLK, OC], F16, name="vh_t")   # [l, lk, oc]
    ot_t, _free_ot = tc.tile([128, NPAIR, L], F16, name="ot_t")  # [o, pair, lq]

    # ---- input DMAs -----------------------------------------------------
    # Order: V weights + first two vblk pairs (the head's V groups and the
    # tensor warm-up), then the K/Q path the attention stream needs first,
    # then the remaining vblk pairs (consumed as in-stream filler), qT's
    # second half, and wo.
    w_pool = tc.alloc_tile_pool(name="wp", bufs=1)
    wv = w_pool.tile([128, ND, OC], F16, tag="wv", name="wv")
    nc.sync.dma_start(out=wv[:], in_=wvT)

    ks_pool = tc.alloc_tile_pool(name="ks", bufs=1)
    qs_pool = tc.alloc_tile_pool(name="qs", bufs=1)
    vs_pool = tc.alloc_tile_pool(name="vs", bufs=4)

    vblks = {}

    def dma_vblk(g):
        vblk = vs_pool.tile([128, ND, 256], F16, tag="vstream",
                            name=f"vblk{g}")
        nc.sync.dma_start(out=vblk[:], in_=vT[g])
        vblks[g] = vblk

    def v_units(lt):
        """Yield one closure per matmul of the V projection for l-tile lt
        (8 dt steps into a pj tile), then the PSUM->SBUF cast."""
        state = {}

        def unit(dt):
            vblk = vblks[lt // 2]
            if dt == 0:
                state["ps"] = pj_pool.tile([128, 512], F32, tag="pj",
                                           name=f"vps{lt}")
            nc.tensor.matmul(state["ps"][:], vblk[:, dt, ts(lt % 2, 128)],
                             wv[:, dt, :],
                             start=(dt == 0), stop=(dt == ND - 1))
            if dt == ND - 1:
                nc.vector.tensor_copy(vh_t[:, lt, :], state["ps"][:])

        for dt in range(ND):
            yield lambda dt=dt: unit(dt)

    # head V groups: l-tiles 0-3 (vblk pairs 0-1)
    dma_vblk(0)
    dma_vblk(1)
    for lt in range(4):
        for u in v_units(lt):
            u()

    # K/Q path
    wk = w_pool.tile([128, ND, OC], F16, tag="wk", name="wk")
    nc.sync.dma_start(out=wk[:], in_=wkT)
    wq = w_pool.tile([128, ND, OC], F16, tag="wq", name="wq")
    nc.sync.dma_start(out=wq[:], in_=wqT)
    # kT/qT land in 512-column slices so the first K0/Q0 groups start as
    # soon as their slice arrives instead of waiting for the full tensor.
    kfull = ks_pool.tile([128, ND, L], F16, tag="kstream", name="kfull")
    qfull = qs_pool.tile([128, ND, L], F16, tag="qstream", name="qfull")

    def dma_k(h):
        nc.sync.dma_start(out=kfull[:, :, ts(h, 512)], in_=kT[h])

    def dma_q(h):
        nc.sync.dma_start(out=qfull[:, :, ts(h, 512)], in_=qT[h])

    dma_k(0)
    dma_k(1)
    dma_q(0)
    dma_vblk(2)
    dma_k(2)
    dma_k(3)
    dma_vblk(3)
    dma_q(1)
    dma_vblk(4)
    dma_vblk(5)
    dma_vblk(6)
    dma_vblk(7)
    dma_q(2)
    dma_q(3)
    wo = w_pool.tile([128, NPAIR, D], F16, tag="wo", name="wo")
    nc.sync.dma_start(out=wo[:], in_=woT)

    # ---- K/Q projections: unit-granular emitters ------------------------
    kh_tiles = {}
    qh_tiles = {}

    def kq_units(which, p):
        """Yield one closure per matmul for the K or Q projection of pair p
        (4 lq-groups x 8 dt), each group accumulating in a pj_pool tile,
        with the PSUM->SBUF cast after each group."""
        src = kfull if which == "k" else qfull
        w = wk if which == "k" else wq
        dsts = kh_tiles if which == "k" else qh_tiles
        pool = kh_pool if which == "k" else qh_pool
        state = {}

        def unit(lq, dt, w=w, src=src):
            if dt == 0:
                if p not in dsts:
                    dsts[p] = pool.tile([128, L], F16, tag=which,
                                        name=f"{which}h{p}")
                state["ps"] = pj_pool.tile([128, 512], F32, tag="pj",
                                           name=f"{which}ps{p}_{lq}")
            nc.tensor.matmul(state["ps"][:], w[:, dt, ts(p, 128)],
                             src[:, dt, ts(lq, 512)],
                             start=(dt == 0), stop=(dt == ND - 1))
            if dt == ND - 1:
                # on DVE: a cast on the scalar engine would delay the exp
                # stream, which the paired S^T buffers depend on
                nc.vector.tensor_copy(dsts[p][:, ts(lq, 512)],
                                      state["ps"][:])

        for lq in range(4):
            for dt in range(ND):
                yield lambda lq=lq, dt=dt: unit(lq, dt)

    def oproj_units(lt):
        """Yield closures for the output projection of l-tile lt: two
        512-column half-groups, each contracting the 4 pairs, then the
        PSUM->SBUF copy and the output DMA."""
        state = {}

        def unit(mb, p):
            if p == 0:
                state[mb] = pj_pool.tile([128, 512], F32, tag="pj",
                                         name=f"ops{lt}_{mb}")
            nc.tensor.matmul(state[mb][:], ot_t[:, p, ts(lt, 128)],
                             wo[:, p, ts(mb, 512)],
                             start=(p == 0), stop=(p == NPAIR - 1))
            if p == NPAIR - 1:
                if "oc" not in state:
                    state["oc"] = oc_pool.tile([128, 1024], F32, tag="oc",
                                               name=f"oc{lt}")
                nc.vector.tensor_copy(state["oc"][:, ts(mb, 512)],
                                      state[mb][:])
                if mb == 1:
                    nc.sync.dma_start(out=out[ts(lt, 128), :],
                                      in_=state["oc"][:])

        for mb in range(2):
            for p in range(NPAIR):
                yield lambda mb=mb, p=p: unit(mb, p)

    # Block order: pairs 0,1 sequential, then pairs 2/3 interleaved per
    # lq-block so output-projection groups release early enough to keep the
    # tensor engine fed through the end of the stream.
    BLOCKS = ([(0, b) for b in range(4)] + [(1, b) for b in range(4)]
              + [(2, 0), (3, 0), (2, 1), (3, 1),
                 (2, 2), (3, 2), (2, 3), (3, 3)])

    # Head: only the first lq-group of K0 and Q0 (enough for S^T[0..3]);
    # everything else streams as deadline-ordered filler.
    k0_units = list(kq_units("k", 0))
    q0_units = list(kq_units("q", 0))
    for u in k0_units[:8]:
        u()
    for u in q0_units[:8]:
        u()

    # Early queue at 8 units/iter: K0's remaining groups (kh cols 512+ are
    # needed from gi 4/8/12) interleaved with V l-tiles (vh[lt] by gi lt+4)
    # and Q0's second group (gi 16).
    v_units_all = []
    for lt in range(4, NLK):
        v_units_all.append(list(v_units(lt)))
    v_q = deque(
        k0_units[8:16] + k0_units[16:24] + v_units_all[0]
        + k0_units[24:32] + v_units_all[1] + q0_units[8:16])
    for g in v_units_all[2:]:
        v_q.extend(g)
    kq_q = deque(q0_units[16:])
    for p in range(1, NPAIR):
        for which in ("k", "q"):
            kq_q.extend(kq_units(which, p))
    op_q = deque()          # released progressively
    op_release = {}         # gi -> list of units
    for B in range(NLQB):
        units = []
        for t in range(4):
            units.extend(oproj_units(4 * B + t))
        # (3, B) is block index 9 + 2B; its drain is emitted at gi
        # (9+2B)*16 + 19 (AV runs four iterations behind), so release
        # one iteration later.
        op_release[(9 + 2 * B) * 16 + 18] = units

    # ---- fused attention stream ----------------------------------------
    pt_tiles = {}
    pta_tiles = {}
    av_tiles = {}
    dn_tiles = {}

    def emit_st_exp(gi):
        block, lk = gi // 16, gi % 16
        p, lqb = BLOCKS[block]
        kh = kh_tiles[p]
        qh = qh_tiles[p]
        st = st_pool.tile([128, 1024], F32, tag="st", name=f"st{gi}")
        nc.tensor.matmul(st[:, 0:512], kh[0:64, ts(lk, 128)],
                         qh[0:64, ts(lqb, LQB)], start=True, stop=True,
                         tile_position=(0, 0))
        nc.tensor.matmul(st[:, 512:1024], kh[64:128, ts(lk, 128)],
                         qh[64:128, ts(lqb, LQB)], start=True, stop=True,
                         tile_position=(64, 0))
        if SCH_MOD and gi % SCH_MOD == 2:
            # Offload ~1/3 of the exps to the vector engine: Schraudolph's
            # trick -- int16(s*1024*log2(e) + (15360-44)) read as an fp16 bit
            # pattern approximates e^s to ~2%; one fused mult+add+convert op.
            pti = pt_pool.tile([128, 1024], I16, tag="pt", name=f"pt{gi}")
            nc.vector.tensor_scalar(pti[:], st[:], SCH_A, SCH_B,
                                    mybir.AluOpType.mult,
                                    mybir.AluOpType.add)
            pt_tiles[gi] = pti[:].bitcast(F16)
        else:
            pt = pt_pool.tile([128, 1024], F16, tag="pt", name=f"pt{gi}")
            nc.scalar.activation(pt[:], st[:], EXP)
            pt_tiles[gi] = pt[:]
        if gi % 2 == 1:
            # fp16 pair-sum of the two pt tiles; the denominator matmuls run
            # once per pair on this, halving their tensor-engine cost.
            pta = pa_pool.tile([128, 1024], F16, tag="pa", name=f"pa{gi}")
            nc.vector.tensor_add(pta[:], pt_tiles[gi - 1], pt_tiles[gi])
            pta_tiles[gi] = pta

    def emit_av_dn(gi):
        block, lk = gi // 16, gi % 16
        p, _lqb = BLOCKS[block]
        if lk == 0:
            av_tiles[block] = av_pool.tile([128, 512], F32, tag="av",
                                           name=f"av{block}")
            dn_tiles[block] = dn_pool.tile([128, 512], F32, tag="dn",
                                           name=f"dn{block}")
        av = av_tiles[block]
        dn = dn_tiles[block]
        pt = pt_tiles.pop(gi)
        nc.tensor.matmul(av[0:64, :], vh_t[:, lk, ts(2 * p, 64)],
                         pt[:, 0:512], start=(lk == 0),
                         stop=(lk == NLK - 1), tile_position=(0, 0),
                         skip_group_check=True)
        nc.tensor.matmul(av[64:128, :], vh_t[:, lk, ts(2 * p + 1, 64)],
                         pt[:, 512:1024], start=(lk == 0),
                         stop=(lk == NLK - 1), tile_position=(0, 64),
                         skip_group_check=True)
        if gi % 2 == 1:
            pta = pta_tiles.pop(gi)
            nc.tensor.matmul(dn[0:64, :], ones_t[:], pta[:, 0:512],
                             start=(lk == 1), stop=(lk == NLK - 1),
                             tile_position=(0, 0), skip_group_check=True)
            nc.tensor.matmul(dn[64:128, :], ones_t[:], pta[:, 512:1024],
                             start=(lk == 1), stop=(lk == NLK - 1),
                             tile_position=(0, 64), skip_group_check=True)

    def emit_drain(block):
        p, lqb = BLOCKS[block]
        av = av_tiles.pop(block)
        dn = dn_tiles.pop(block)
        rc = rc_pool.tile([128, 512], F32, tag="rc", name=f"rc{block}")
        nc.vector.reciprocal_approx_fast(out=rc[:], in_=dn[:])
        nc.vector.tensor_mul(ot_t[:, p, ts(lqb, LQB)], av[:, :], rc[:, :])

    next_av = 0
    for gi in range(NITER + 4):
        if gi in op_release:
            op_q.extend(op_release.pop(gi))
        if gi < NITER:
            emit_st_exp(gi)
            # filler pacing: V first (block 0 consumes it immediately, at a
            # high rate), then K/Q projections, then output projection.
            # K/Q and o-proj fills cluster on even (dn-free) iterations:
            # fewer PE tile_position switches per iteration pair.
            if v_q:
                npop = 8
            elif gi % 2 == 1:
                npop = 0
            elif gi < 120:
                npop = 4
            else:
                npop = 3
            filled = 0
            for _ in range(npop):
                if v_q:
                    v_q.popleft()()
                    filled += 1
                elif kq_q:
                    kq_q.popleft()()
                    filled += 1
                elif op_q:
                    op_q.popleft()()
                    filled += 1
        while next_av < NITER and (
                next_av <= gi - 4
                or (next_av <= gi - 2 and next_av % 16 >= 14)):
            emit_av_dn(next_av)
            if next_av % 16 == 15:
                emit_drain(next_av // 16)
            next_av += 1

    # ---- tail: remaining output projection ------------------------------
    for rel in sorted(op_release):
        op_q.extend(op_release.pop(rel))
    while op_q:
        op_q.popleft()()

    for pool in (vs_pool, qs_pool, ks_pool, w_pool):
        pool.release()
    _free_ot(); _free_vh(); _free_wrm(); _free_ones()
    for pool in (oc_pool, rc_pool, pa_pool, pt_pool, qh_pool, kh_pool,
                 dn_pool, av_pool, pj_pool, st_pool):
        pool.release()


_CACHED_NC = None


def _get_program():
    global _CACHED_NC
    if _CACHED_NC is None:
        _CACHED_NC = build_program()
    return _CACHED_NC


def make_in_maps(q, k, v, w_q, w_k, w_v, w_o):
    in_maps = []
    for c in range(N_CORES):
        b, half = c // 2, c % 2
        osl = slice(half * OC, (half + 1) * OC)
        def tile_qk(xT):
            # [D, L] -> [slice h, p, n, 512]: [p] contiguous 8KB runs
            return np.ascontiguousarray(
                xT.reshape(8, 128, 4, 512).transpose(2, 1, 0, 3)
            ).astype(np.float16)

        def tile_w(wT, n):
            # [D, cols] -> [p, n, cols]: contiguous per partition
            return np.ascontiguousarray(
                wT.reshape(n, 128, -1).transpose(1, 0, 2)).astype(np.float16)

        in_maps.append({
            "qT": tile_qk(q[b].T),
            "kT": tile_qk(k[b].T),
            "vT": np.ascontiguousarray(
                v[b].T.reshape(8, 128, 8, 256).transpose(2, 1, 0, 3)
            ).astype(np.float16),
            # temperature sqrt(d_k)=8 folded into the Q weights
            "wqT": tile_w(w_q[osl].T / 8.0, 8),
            "wkT": tile_w(w_k[osl].T, 8),
            "wvT": tile_w(w_v[osl].T, 8),
            "woT": tile_w(w_o[:, osl].T, 4),
        })
    return in_maps


def run_on_hw(q, k, v, w_q, w_k, w_v, w_o, trace=False, **trace_kwargs):
    from concourse.bass_utils import run_bass_kernel_spmd
    nc = _get_program()
    in_maps = make_in_maps(q, k, v, w_q, w_k, w_v, w_o)
    res = run_bass_kernel_spmd(nc, in_maps, core_ids=list(range(N_CORES)),
                               trace=trace, **trace_kwargs)
    B = 4
    outp = np.empty((B, L, D), np.float32)
    for b in range(B):
        outp[b] = res.results[2 * b]["out"] + res.results[2 * b + 1]["out"]
    return outp, res


def _numpy_fallback(q, k, v, w_q, w_k, w_v, w_o, mask):
    NEG = -1000000000.0
    B = q.shape[0]
    outs = []
    for b in range(B):
        qh = (q[b] @ w_q.T).reshape(L, 16, 64).transpose(1, 0, 2)
        kh = (k[b] @ w_k.T).reshape(L, 16, 64).transpose(1, 0, 2)
        vh = (v[b] @ w_v.T).reshape(L, 16, 64).transpose(1, 0, 2)
        s = np.einsum("hqd,hkd->hqk", qh / 8.0, kh)
        s = np.where(mask[b][None] == 0, NEG, s)
        s = s - s.max(axis=-1, keepdims=True)
        p = np.exp(s)
        p /= p.sum(axis=-1, keepdims=True)
        o = np.einsum("hqk,hkd->hqd", p, vh)
        o = o.transpose(1, 0, 2).reshape(L, D)
        outs.append(o @ w_o.T)
    return np.stack(outs).astype(np.float32)


def kernel(q, k, v, w_q, w_k, w_v, w_o, mask):
    q = np.asarray(q, np.float32)
    k = np.asarray(k, np.float32)
    v = np.asarray(v, np.float32)
    w_q = np.asarray(w_q, np.float32)
    w_k = np.asarray(w_k, np.float32)
    w_v = np.asarray(w_v, np.float32)
    w_o = np.asarray(w_o, np.float32)
    mask = np.asarray(mask)
    if not np.all(mask != 0):
        # never hit with the spec'd all-ones mask; correctness fallback
        return _numpy_fallback(q, k, v, w_q, w_k, w_v, w_o, mask)
    outp, _ = run_on_hw(q, k, v, w_q, w_k, w_v, w_o)
    return outp


# revision 66
# speedup vs baseline: 1.0101x; 1.0101x over previous
"""Multi-head attention (B=4, L=2048, D=1024, H=16) on 8 Trainium2 NeuronCores.

Sharding: core c handles batch b=c//2 and head-half half=c%2 (8 heads = 512 of
the 1024 projection output dims).  Each core computes its heads' Q/K/V
projections, attention, and a full-L partial of the output projection
(contracting only its 512 head dims).  The host sums the two partials per batch
(the "all-reduce after fc" done at gather time).

Single fused pipeline: the 256-iteration attention stream (4 head-pairs x
4 lq-blocks x 16 lk-tiles) is emitted as one continuous instruction stream.
S^T matmuls run two iterations ahead of the AV/denominator matmuls so the
scalar engine's exp activations stream back-to-back; K/Q projections of later
pairs and output-projection groups are interleaved one matmul per iteration as
filler so the tensor engine never idles (holding its high p-state).

PSUM (8 banks): 2 x st[128,1024]f32 (4) + 2 x proj[128,512]f32 (2) +
av[128,512]f32 (1) + dn[128,512]f32 (1).
"""

import sys
from collections import deque

import numpy as np

if "/opt/trn_rl_repo" not in sys.path:
    sys.path.insert(0, "/opt/trn_rl_repo")

import concourse.bass as bass
import concourse.mybir as mybir
from concourse import bacc
import concourse.tile as tile
from concourse.bass import ts

F32 = mybir.dt.float32
F16 = mybir.dt.float16
I16 = mybir.dt.int16
EXP = mybir.ActivationFunctionType.Exp
# Schraudolph fp16 exp: int16(s * 1024/ln2 + (15360 - 44)) viewed as fp16
SCH_A = 1024.0 / float(np.log(2.0))
SCH_B = 15360.0 - 44.0
# Offload exp of iterations gi%SCH_MOD==2 to the vector engine (0 = off)
SCH_MOD = 0

L = 2048          # sequence length
D = 1024          # model dim
OC = 512          # output-projection dims owned by one core (8 heads x 64)
NPAIR = 4         # head pairs per core (pair = 128 projection dims)
NLQB = 4          # lq blocks of 512
LQB = 512
NLK = 16          # lk tiles of 128
ND = 8            # d-model tiles of 128
N_CORES = 8
NITER = NPAIR * NLQB * NLK  # 256


def build_program():
    nc = bacc.Bacc("TRN2", debug=False, enable_asserts=False,
                   target_bir_lowering=False)

    # Inputs arrive HOST-PRE-TILED so every DMA source is contiguous per
    # partition (4-8KB descriptor runs instead of 0.5-1KB): qT/kT as
    # [slice, p, n, 512], vT as [vblk, p, n, 256], weights as [p, n, cols].
    qT = nc.dram_tensor("qT", [4, 128, ND, 512], F16,
                        kind="ExternalInput").ap()
    kT = nc.dram_tensor("kT", [4, 128, ND, 512], F16,
                        kind="ExternalInput").ap()
    vT = nc.dram_tensor("vT", [8, 128, ND, 256], F16,
                        kind="ExternalInput").ap()
    wqT = nc.dram_tensor("wqT", [128, ND, OC], F16,
                         kind="ExternalInput").ap()
    wkT = nc.dram_tensor("wkT", [128, ND, OC], F16,
                         kind="ExternalInput").ap()
    wvT = nc.dram_tensor("wvT", [128, ND, OC], F16,
                         kind="ExternalInput").ap()
    woT = nc.dram_tensor("woT", [128, NPAIR, D], F16,
                         kind="ExternalInput").ap()
    out = nc.dram_tensor("out", [L, D], F32, kind="ExternalOutput").ap()

    with tile.TileContext(nc, pool_alloc_mode="queue") as tc:
        build_body(nc, tc, qT, kT, vT, wqT, wkT, wvT, woT, out)
    nc.compile()
    return nc


def build_body(nc, tc, qT, kT, vT, wqT, wkT, wvT, woT, out):
    # ---- pools ----------------------------------------------------------
    st_pool = tc.alloc_tile_pool(name="st", bufs=2, space="PSUM")   # 4 banks
    pj_pool = tc.alloc_tile_pool(name="pj", bufs=2, space="PSUM")   # 2 banks
    av_pool = tc.alloc_tile_pool(name="av", bufs=1, space="PSUM")   # 1 bank
    dn_pool = tc.alloc_tile_pool(name="dn", bufs=1, space="PSUM")   # 1 bank

    kh_pool = tc.alloc_tile_pool(name="kh", bufs=3)
    qh_pool = tc.alloc_tile_pool(name="qh", bufs=3)
    pt_pool = tc.alloc_tile_pool(name="pt", bufs=7)
    pa_pool = tc.alloc_tile_pool(name="pa", bufs=4)
    rc_pool = tc.alloc_tile_pool(name="rc", bufs=2)
    oc_pool = tc.alloc_tile_pool(name="ocp", bufs=2)

    ones_t, _free_ones = tc.tile([128, 64], F16, name="ones_t")
    nc.vector.memset(ones_t[:], 1.0)
    wrm_t, _free_wrm = tc.tile([128, 512], F16, name="wrm_t")
    nc.vector.memset(wrm_t[:], 0.0)

    # Warm the tensor engine while the first input DMAs are in flight: its
    # clock ramps only under sustained execution (2.4GHz after ~3us of
    # continuous busy).  N must be large -- narrow matmuls are decode-bound
    # and barely accumulate busy time.  Sized to bridge only the ~5us until
    # the first (pre-tiled, fast) V-input DMA lands: more warm-up would
    # delay the V projection behind it in tensor queue order.
    for wu in range(8):
        psb = pj_pool.tile([128, 512], F32, tag="pj", name=f"wu{wu}")
        nc.tensor.matmul(psb[0:64, :], ones_t[:], wrm_t[:],
                         start=True, stop=True)

    vh_t, _free_vh = tc.tile([128, NLK, OC], F16, name="vh_t")   # [l, lk, oc]
    ot_t, _free_ot = tc.tile([128, NPAIR, L], F16, name="ot_t")  # [o, pair, lq]

    # ---- input DMAs -----------------------------------------------------
    # Order: V weights + first two vblk pairs (the head's V groups and the
    # tensor warm-up), then the K/Q path the attention stream needs first,
    # then the remaining vblk pairs (consumed as in-stream filler), qT's
    # second half, and wo.
    w_pool = tc.alloc_tile_pool(name="wp", bufs=1)
    wv = w_pool.tile([128, ND, OC], F16, tag="wv", name="wv")
    nc.sync.dma_start(out=wv[:], in_=wvT)

    ks_pool = tc.alloc_tile_pool(name="ks", bufs=1)
    qs_pool = tc.alloc_tile_pool(name="qs", bufs=1)
    vs_pool = tc.alloc_tile_pool(name="vs", bufs=4)

    vblks = {}

    def dma_vblk(g):
        vblk = vs_pool.tile([128, ND, 256], F16, tag="vstream",
                            name=f"vblk{g}")
        nc.sync.dma_start(out=vblk[:], in_=vT[g])
        vblks[g] = vblk

    def v_units(lt):
        """Yield one closure per matmul of the V projection for l-tile lt
        (8 dt steps into a pj tile), then the PSUM->SBUF cast."""
        state = {}

        def unit(dt):
            vblk = vblks[lt // 2]
            if dt == 0:
                state["ps"] = pj_pool.tile([128, 512], F32, tag="pj",
                                           name=f"vps{lt}")
            nc.tensor.matmul(state["ps"][:], vblk[:, dt, ts(lt % 2, 128)],
                             wv[:, dt, :],
                             start=(dt == 0), stop=(dt == ND - 1))
            if dt == ND - 1:
                nc.vector.tensor_copy(vh_t[:, lt, :], state["ps"][:])

        for dt in range(ND):
            yield lambda dt=dt: unit(dt)

    # head V groups: l-tiles 0-3 (vblk pairs 0-1)
    dma_vblk(0)
    dma_vblk(1)
    for lt in range(4):
        for u in v_units(lt):
            u()

    # K/Q path
    wk = w_pool.tile([128, ND, OC], F16, tag="wk", name="wk")
    nc.sync.dma_start(out=wk[:], in_=wkT)
    wq = w_pool.tile([128, ND, OC], F16, tag="wq", name="wq")
    nc.sync.dma_start(out=wq[:], in_=wqT)
    # kT/qT land in 512-column slices so the first K0/Q0 groups start as
    # soon as their slice arrives instead of waiting for the full tensor.
    kfull = ks_pool.tile([128, ND, L], F16, tag="kstream", name="kfull")
    qfull = qs_pool.tile([128, ND, L], F16, tag="qstream", name="qfull")

    def dma_k(h):
        nc.sync.dma_start(out=kfull[:, :, ts(h, 512)], in_=kT[h])

    def dma_q(h):
        nc.sync.dma_start(out=qfull[:, :, ts(h, 512)], in_=qT[h])

    dma_k(0)
    dma_k(1)
    dma_q(0)
    dma_vblk(2)
    dma_k(2)
    dma_k(3)
    dma_vblk(3)
    dma_q(1)
    dma_vblk(4)
    dma_vblk(5)
    dma_vblk(6)
    dma_vblk(7)
    dma_q(2)
    dma_q(3)
    wo = w_pool.tile([128, NPAIR, D], F16, tag="wo", name="wo")
    nc.sync.dma_start(out=wo[:], in_=woT)

    # ---- K/Q projections: unit-granular emitters ------------------------
    kh_tiles = {}
    qh_tiles = {}

    def kq_units(which, p):
        """Yield one closure per matmul for the K or Q projection of pair p
        (4 lq-groups x 8 dt), each group accumulating in a pj_pool tile,
        with the PSUM->SBUF cast after each group."""
        src = kfull if which == "k" else qfull
        w = wk if which == "k" else wq
        dsts = kh_tiles if which == "k" else qh_tiles
        pool = kh_pool if which == "k" else qh_pool
        state = {}

        def unit(lq, dt, w=w, src=src):
            if dt == 0:
                if p not in dsts:
                    dsts[p] = pool.tile([128, L], F16, tag=which,
                                        name=f"{which}h{p}")
                state["ps"] = pj_pool.tile([128, 512], F32, tag="pj",
                                           name=f"{which}ps{p}_{lq}")
            nc.tensor.matmul(state["ps"][:], w[:, dt, ts(p, 128)],
                             src[:, dt, ts(lq, 512)],
                             start=(dt == 0), stop=(dt == ND - 1))
            if dt == ND - 1:
                # on DVE: a cast on the scalar engine would delay the exp
                # stream, which the paired S^T buffers depend on
                nc.vector.tensor_copy(dsts[p][:, ts(lq, 512)],
                                      state["ps"][:])

        for lq in range(4):
            for dt in range(ND):
                yield lambda lq=lq, dt=dt: unit(lq, dt)

    def oproj_units(lt):
        """Yield closures for the output projection of l-tile lt: two
        512-column half-groups, each contracting the 4 pairs, then the
        PSUM->SBUF copy and the output DMA."""
        state = {}

        def unit(mb, p):
            if p == 0:
                state[mb] = pj_pool.tile([128, 512], F32, tag="pj",
                                         name=f"ops{lt}_{mb}")
            nc.tensor.matmul(state[mb][:], ot_t[:, p, ts(lt, 128)],
                             wo[:, p, ts(mb, 512)],
                             start=(p == 0), stop=(p == NPAIR - 1))
            if p == NPAIR - 1:
                if "oc" not in state:
                    state["oc"] = oc_pool.tile([128, 1024], F32, tag="oc",
                                               name=f"oc{lt}")
                nc.vector.tensor_copy(state["oc"][:, ts(mb, 512)],
                                      state[mb][:])
                if mb == 1:
                    nc.sync.dma_start(out=out[ts(lt, 128), :],
                                      in_=state["oc"][:])

        for mb in range(2):
            for p in range(NPAIR):
                yield lambda mb=mb, p=p: unit(mb, p)

    # Block order: pairs 0,1 sequential, then pairs 2/3 interleaved per
    # lq-block so output-projection groups release early enough to keep the
    # tensor engine fed through the end of the stream.
    BLOCKS = ([(0, b) for b in range(4)] + [(1, b) for b in range(4)]
              + [(2, 0), (3, 0), (2, 1), (3, 1),
                 (2, 2), (3, 2), (2, 3), (3, 3)])

    # Head: only the first lq-group of K0 and Q0 (enough for S^T[0..3]);
    # everything else streams as deadline-ordered filler.
    k0_units = list(kq_units("k", 0))
    q0_units = list(kq_units("q", 0))
    for u in k0_units[:8]:
        u()
    for u in q0_units[:8]:
        u()

    # Early queue at 8 units/iter: K0's remaining groups (kh cols 512+ are
    # needed from gi 4/8/12) interleaved with V l-tiles (vh[lt] by gi lt+4)
    # and Q0's second group (gi 16).
    v_units_all = []
    for lt in range(4, NLK):
        v_units_all.append(list(v_units(lt)))
    v_q = deque(
        k0_units[8:16] + k0_units[16:24] + v_units_all[0]
        + k0_units[24:32] + v_units_all[1] + q0_units[8:16])
    for g in v_units_all[2:]:
        v_q.extend(g)
    kq_q = deque(q0_units[16:])
    for p in range(1, NPAIR):
        for which in ("k", "q"):
            kq_q.extend(kq_units(which, p))
    op_q = deque()          # released progressively
    op_release = {}         # gi -> list of units
    for B in range(NLQB):
        units = []
        for t in range(4):
            units.extend(oproj_units(4 * B + t))
        # (3, B) is block index 9 + 2B; its drain is emitted at gi
        # (9+2B)*16 + 19 (AV runs four iterations behind), so release
        # one iteration later.
        op_release[(9 + 2 * B) * 16 + 18] = units

    # ---- fused attention stream ----------------------------------------
    pt_tiles = {}
    pta_tiles = {}
    av_tiles = {}
    dn_tiles = {}

    def emit_st_exp(gi):
        block, lk = gi // 16, gi % 16
        p, lqb = BLOCKS[block]
        kh = kh_tiles[p]
        qh = qh_tiles[p]
        st = st_pool.tile([128, 1024], F32, tag="st", name=f"st{gi}")
        nc.tensor.matmul(st[:, 0:512], kh[0:64, ts(lk, 128)],
                         qh[0:64, ts(lqb, LQB)], start=True, stop=True,
                         tile_position=(0, 0))
        nc.tensor.matmul(st[:, 512:1024], kh[64:128, ts(lk, 128)],
                         qh[64:128, ts(lqb, LQB)], start=True, stop=True,
                         tile_position=(64, 0))
        if SCH_MOD and gi % SCH_MOD == 2:
            # Offload ~1/3 of the exps to the vector engine: Schraudolph's
            # trick -- int16(s*1024*log2(e) + (15360-44)) read as an fp16 bit
            # pattern approximates e^s to ~2%; one fused mult+add+convert op.
            pti = pt_pool.tile([128, 1024], I16, tag="pt", name=f"pt{gi}")
            nc.vector.tensor_scalar(pti[:], st[:], SCH_A, SCH_B,
                                    mybir.AluOpType.mult,
                                    mybir.AluOpType.add)
            pt_tiles[gi] = pti[:].bitcast(F16)
        else:
            pt = pt_pool.tile([128, 1024], F16, tag="pt", name=f"pt{gi}")
            nc.scalar.activation(pt[:], st[:], EXP)
            pt_tiles[gi] = pt[:]
        if gi % 2 == 1:
            # fp16 pair-sum of the two pt tiles; the denominator matmuls run
            # once per pair on this, halving their tensor-engine cost.
            pta = pa_pool.tile([128, 1024], F16, tag="pa", name=f"pa{gi}")
            nc.vector.tensor_add(pta[:], pt_tiles[gi - 1], pt_tiles[gi])
            pta_tiles[gi] = pta

    def emit_av_dn(gi):
        block, lk = gi // 16, gi % 16
        p, _lqb = BLOCKS[block]
        if lk == 0:
            av_tiles[block] = av_pool.tile([128, 512], F32, tag="av",
                                           name=f"av{block}")
            dn_tiles[block] = dn_pool.tile([128, 512], F32, tag="dn",
                                           name=f"dn{block}")
        av = av_tiles[block]
        dn = dn_tiles[block]
        pt = pt_tiles.pop(gi)
        nc.tensor.matmul(av[0:64, :], vh_t[:, lk, ts(2 * p, 64)],
                         pt[:, 0:512], start=(lk == 0),
                         stop=(lk == NLK - 1), tile_position=(0, 0),
                         skip_group_check=True)
        nc.tensor.matmul(av[64:128, :], vh_t[:, lk, ts(2 * p + 1, 64)],
                         pt[:, 512:1024], start=(lk == 0),
                         stop=(lk == NLK - 1), tile_position=(0, 64),
                         skip_group_check=True)
        if gi % 2 == 1:
            pta = pta_tiles.pop(gi)
            nc.tensor.matmul(dn[0:64, :], ones_t[:], pta[:, 0:512],
                             start=(lk == 1), stop=(lk == NLK - 1),
                             tile_position=(0, 0), skip_group_check=True)
            nc.tensor.matmul(dn[64:128, :], ones_t[:], pta[:, 512:1024],
                             start=(lk == 1), stop=(lk == NLK - 1),
                             tile_position=(0, 64), skip_group_check=True)

    def emit_drain(block):
        p, lqb = BLOCKS[block]
        av = av_tiles.pop(block)
        dn = dn_tiles.pop(block)
        rc = rc_pool.tile([128, 512], F32, tag="rc", name=f"rc{block}")
        nc.vector.reciprocal_approx_fast(out=rc[:], in_=dn[:])
        nc.vector.tensor_mul(ot_t[:, p, ts(lqb, LQB)], av[:, :], rc[:, :])

    next_av = 0
    for gi in range(NITER + 4):
        if gi in op_release:
            op_q.extend(op_release.pop(gi))
        if gi < NITER:
            emit_st_exp(gi)
            # filler pacing: V first (block 0 consumes it immediately, at a
            # high rate), then K/Q projections, then output projection.
            # K/Q and o-proj fills cluster on even (dn-free) iterations:
            # fewer PE tile_position switches per iteration pair.
            if v_q:
                npop = 8
            elif gi % 2 == 1:
                npop = 0
            elif gi < 120:
                npop = 4
            else:
                npop = 3
            filled = 0
            for _ in range(npop):
                if v_q:
                    v_q.popleft()()
                    filled += 1
                elif kq_q:
                    kq_q.popleft()()
                    filled += 1
                elif op_q:
                    op_q.popleft()()
                    filled += 1
        while next_av < NITER and (
                next_av <= gi - 4
                or (next_av <= gi - 2 and next_av % 16 >= 14)):
            emit_av_dn(next_av)
            if next_av % 16 == 15:
                emit_drain(next_av // 16)
            next_av += 1

    # ---- tail: remaining output projection ------------------------------
    for rel in sorted(op_release):
        op_q.extend(op_release.pop(rel))
    while op_q:
        op_q.popleft()()

    for pool in (vs_pool, qs_pool, ks_pool, w_pool):
        pool.release()
    _free_ot(); _free_vh(); _free_wrm(); _free_ones()
    for pool in (oc_pool, rc_pool, pa_pool, pt_pool, qh_pool, kh_pool,
                 dn_pool, av_pool, pj_pool, st_pool):
        pool.release()


_CACHED_NC = None


def _get_program():
    global _CACHED_NC
    if _CACHED_NC is None:
        _CACHED_NC = build_program()
    return _CACHED_NC


def make_in_maps(q, k, v, w_q, w_k, w_v, w_o):
    in_maps = []
    for c in range(N_CORES):
        b, half = c // 2, c % 2
        osl = slice(half * OC, (half + 1) * OC)
        def tile_qk(xT):
            # [D, L] -> [slice h, p, n, 512]: [p] contiguous 8KB runs
            return np.ascontiguousarray(
                xT.reshape(8, 128, 4, 512).transpose(2, 1, 0, 3)
            ).astype(np.float16)

        def tile_w(wT, n):
            # [D, cols] -> [p, n, cols]: contiguous per partition
            return np.ascontiguousarray(
                wT.reshape(n, 128, -1).transpose(1, 0, 2)).astype(np.float16)

        in_maps.append({
            "qT": tile_qk(q[b].T),
            "kT": tile_qk(k[b].T),
            "vT": np.ascontiguousarray(
                v[b].T.reshape(8, 128, 8, 256).transpose(2, 1, 0, 3)
            ).astype(np.float16),
            # temperature sqrt(d_k)=8 folded into the Q weights
            "wqT": tile_w(w_q[osl].T / 8.0, 8),
            "wkT": tile_w(w_k[osl].T, 8),
            "wvT": tile_w(w_v[osl].T, 8),
            "woT": tile_w(w_o[:, osl].T, 4),
        })
    return in_maps


def run_on_hw(q, k, v, w_q, w_k, w_v, w_o, trace=False, **trace_kwargs):
    from concourse.bass_utils import run_bass_kernel_spmd
    nc = _get_program()
    in_maps = make_in_maps(q, k, v, w_q, w_k, w_v, w_o)
    res = run_bass_kernel_spmd(nc, in_maps, core_ids=list(range(N_CORES)),
                               trace=trace, **trace_kwargs)
    B = 4
    outp = np.empty((B, L, D), np.float32)
    for b in range(B):
        outp[b] = res.results[2 * b]["out"] + res.results[2 * b + 1]["out"]
    return outp, res


def _numpy_fallback(q, k, v, w_q, w_k, w_v, w_o, mask):
    NEG = -1000000000.0
    B = q.shape[0]
    outs = []
    for b in range(B):
        qh = (q[b] @ w_q.T).reshape(L, 16, 64).transpose(1, 0, 2)
        kh = (k[b] @ w_k.T).reshape(L, 16, 64).transpose(1, 0, 2)
        vh = (v[b] @ w_v.T).reshape(L, 16, 64).transpose(1, 0, 2)
        s = np.einsum("hqd,hkd->hqk", qh / 8.0, kh)
        s = np.where(mask[b][None] == 0, NEG, s)
        s = s - s.max(axis=-1, keepdims=True)
        p = np.exp(s)
        p /= p.sum(axis=-1, keepdims=True)
        o = np.einsum("hqk,hkd->hqd", p, vh)
        o = o.transpose(1, 0, 2).reshape(L, D)
        outs.append(o @ w_o.T)
    return np.stack(outs).astype(np.float32)


def kernel(q, k, v, w_q, w_k, w_v, w_o, mask):
    q = np.asarray(q, np.float32)
    k = np.asarray(k, np.float32)
    v = np.asarray(v, np.float32)
    w_q = np.asarray(w_q, np.float32)
    w_k = np.asarray(w_k, np.float32)
    w_v = np.asarray(w_v, np.float32)
    w_o = np.asarray(w_o, np.float32)
    mask = np.asarray(mask)
    if not np.all(mask != 0):
        # never hit with the spec'd all-ones mask; correctness fallback
        return _numpy_fallback(q, k, v, w_q, w_k, w_v, w_o, mask)
    outp, _ = run_on_hw(q, k, v, w_q, w_k, w_v, w_o)
    return outp
